# revision 25
# baseline (speedup 1.0000x reference)
"""Causal multi-head self-attention (RoPE, V-uses-Q-projection bug preserved)
as a Bass/Tile kernel for 8 Trainium2 NeuronCores — v4.

Sharding: core c -> batch b = c//4, head-group g = c%4 (4 heads of 16).
Each core computes its 4 heads' attention for its batch and a partial
output projection; partials are summed (and bo added) per batch on the host.

v4 changes over v3 (cost-model-guided; 190995ns -> 148197ns):
  - xt pool is 4-deep so no input DMA parks waiting on a ring buffer; late
    input DMAs ride the Pool queue so the Act SEQ stays free for the
    transpose copies that gate the first exp
  - PE p-state warmup: 32 dummy ident matmuls during the initial DMA wait
    ramp the tensor clock to 2.4GHz before the first projection matmul
  - pre-attention software pipeline: proj(tg0) / rope_q0+rope_k0 (DVE/Pool)
    / proj(tg1) / oc0-only transposes; the oc1 (heads 2/3) transposes defer
    into attention(0) as the first prep fillers
  - rope runs on DVE with bf16 cos/sin tables (2x_1p mode)
  - softmax divide: a cross-partition-base reciprocal moves the replicated
    denominator onto the feature rows in one DVE op (no partition-shift DMA)
  - attention emits sc(step i+1) BEFORE the exp-dependent AV(step i) so the
    in-order PE queue always has ready work while Act runs exp
  - two filler queues: prep (proj/rope/transpose of tg2,3 — prerequisites of
    attention(1), drained through attention(0) between sc and AV) and oproj
    (the only deferrable PE work, spread across attention(1)'s Act-bound
    stretches); transposes are queued well after their rope so they never
    head-of-line-block the PE queue
  - oproj chunk 3 (the kernel tail) copies PSUM->SBUF on the then-idle Act
    engine with per-half out DMAs; bo is added on the host
"""

import os
from contextlib import ExitStack

import numpy as np

import concourse.bass as bass
import concourse.mybir as mybir
import concourse.tile as tile
from concourse import bacc
from concourse.bass import ds, ts
from concourse.masks import make_identity

F32 = mybir.dt.float32
BF16 = mybir.dt.bfloat16
AF = mybir.ActivationFunctionType
ALU = mybir.AluOpType

B, T, D, H, DK = 2, 2048, 1024, 16, 64
THETA = 10000.0
NCORE, GPB = 8, 4          # cores; head-groups per batch
HPC = H // GPB             # heads per core = 4
OC = HPC * DK              # per-core projected features = 256
NT = T // 128              # 16 t-tiles
NDC = D // 128             # 8 contraction chunks
NG, GT = 4, 4              # t-groups; t-tiles per group


def build_kernel():
    nc = bacc.Bacc("TRN2", target_bir_lowering=False, debug=False)

    xT = nc.dram_tensor("xT", [D, T], BF16, kind="ExternalInput")
    wqk = nc.dram_tensor("wqk", [D, 2 * OC], BF16, kind="ExternalInput")
    wo = nc.dram_tensor("wo", [OC, D], BF16, kind="ExternalInput")
    bqk = nc.dram_tensor("bqk", [1, 2 * OC], F32, kind="ExternalInput")
    cosT = nc.dram_tensor("cosT", [128, NT * DK], BF16, kind="ExternalInput")
    sinT = nc.dram_tensor("sinT", [128, NT * (DK // 2)], BF16, kind="ExternalInput")
    maskt = nc.dram_tensor("maskt", [128, 128], BF16, kind="ExternalInput")
    out = nc.dram_tensor("out", [T, D], F32, kind="ExternalOutput")

    with tile.TileContext(nc) as tc, ExitStack() as top:
        consts = top.enter_context(tc.tile_pool(name="consts", bufs=1))
        wpool = top.enter_context(tc.tile_pool(name="weights", bufs=1))
        vk = top.enter_context(tc.tile_pool(name="vk", bufs=1))
        qtkt = top.enter_context(tc.tile_pool(name="qtkt", bufs=1))
        heads_pool = top.enter_context(tc.tile_pool(name="heads", bufs=1))

        # ---- input DMAs ----
        # coarse first transfers: proj tile 0 needs ALL dc chunks, and one
        # big DMA completes sooner than 8 serialized HWDGE generations
        wqk_sb = wpool.tile([128, NDC, 2 * OC], BF16)
        wqk_v = wqk.ap().rearrange("(dc p) c -> p dc c", p=128)
        nc.sync.dma_start(wqk_sb[:, 0:4, :], wqk_v[:, 0:4, :])
        nc.sync.dma_start(wqk_sb[:, 4:8, :], wqk_v[:, 4:8, :])

        xT_v = xT.ap().rearrange("(dc p) t -> p dc t", p=128)
        xtp = top.enter_context(tc.tile_pool(name="xt", bufs=4))
        xts = []

        def issue_xt(tg, eng, ndc_slice=1):
            t_ = xtp.tile([128, NDC, 512], BF16, tag="xt", name=f"xt{tg}")
            for s in range(ndc_slice):
                w = NDC // ndc_slice
                eng.dma_start(
                    t_[:, ds(s * w, w), :], xT_v[:, ds(s * w, w), ts(tg, 512)]
                )
            xts.append(t_)

        issue_xt(0, nc.scalar)
        bqk_rep = consts.tile([128, 2 * OC], F32)
        nc.scalar.dma_start(bqk_rep[:], bqk.ap().to_broadcast((128, 2 * OC)))
        issue_xt(1, nc.sync)

        # local compute while DMAs fly
        ident = consts.tile([128, 128], BF16)
        make_identity(nc, ident[:])
        v_sb = vk.tile([128, NT, HPC, 2 * DK], BF16)
        for h in range(HPC):  # ones on the side opposite the V features
            sl = slice(DK, 2 * DK) if h % 2 == 0 else slice(0, DK)
            nc.gpsimd.memset(v_sb[:, :, h, sl], 1.0)
        dume = consts.tile([1, 8], F32)
        nc.vector.memset(dume[:], 0.0)
        dume2 = consts.tile([1, 8], F32)
        nc.scalar.activation(dume2[:], dume[:], AF.Exp)  # preload Exp table

        # remaining inputs ride the Pool queue (dep-free, cheap to issue
        # there, and the Act SEQ stays free for the copies gating exp #0)
        cos_sb = consts.tile([128, NT, DK], BF16)
        nc.gpsimd.dma_start(cos_sb[:], cosT.ap().rearrange("p (j i) -> p j i", j=NT))
        sin_sb = consts.tile([128, NT, DK // 2], BF16)
        nc.gpsimd.dma_start(sin_sb[:], sinT.ap().rearrange("p (j i) -> p j i", j=NT))
        issue_xt(2, nc.gpsimd)
        maskt_sb = consts.tile([128, 128], BF16)
        nc.gpsimd.dma_start(maskt_sb[:], maskt.ap())
        issue_xt(3, nc.gpsimd)
        wo_sb = wpool.tile([128, 2, D], BF16)
        nc.gpsimd.dma_start(wo_sb[:], wo.ap().rearrange("(jc p) d -> p jc d", p=128))

        qt_sb = [
            qtkt.tile([128, T // 2], BF16, tag=f"qt{i}", name=f"qt_sb{i}")
            for i in range(4)
        ]
        kt_sb = [
            qtkt.tile([128, T // 2], BF16, tag=f"kt{i}", name=f"kt_sb{i}")
            for i in range(4)
        ]
        heads_t = [
            heads_pool.tile([128, T // 2], BF16, tag=f"ht{i}", name=f"heads_t{i}")
            for i in range(4)
        ]

        with tc.tile_pool(name="expp", bufs=int(os.environ.get("ET_BUFS", "8"))) as expp, \
             tc.tile_pool(name="divp", bufs=int(os.environ.get("DIV_BUFS", "3"))) as divp, \
             tc.tile_pool(name="outp", bufs=int(os.environ.get("OUTP_BUFS", "4"))) as outp, \
             tc.tile_pool(name="ropep", bufs=2) as ropep, \
             tc.tile_pool(name="psA", bufs=2, space="PSUM") as psA:

            k_sbs = {}

            # PE p-state warmup: the tensor engine only reaches 2.4GHz after
            # 3us of continuous execution (1.2GHz before, 0.65GHz from cold).
            # Dummy ident x ident matmuls during the initial DMA wait ramp the
            # clock so the DMA-paced projection matmuls run at full speed.
            # The warm tile rides the pp ring and is recycled by the first
            # projection tiles.
            warm = psA.tile([128, 512], F32, tag="pp", name="warm")
            for _ in range(64):
                nc.tensor.matmul(warm[:, 0:128], ident[:], ident[:],
                                 start=True, stop=True)

            def proj_tile(tg, tl):
                xt = xts[tg]
                t = GT * tg + tl
                ps = psA.tile([128, 2 * OC], F32, tag="pp", name=f"ps{t}")
                for dc in range(NDC):
                    nc.tensor.matmul(
                        ps[:],
                        xt[:, dc, ts(tl, 128)],
                        wqk_sb[:, dc, :],
                        start=(dc == 0),
                        stop=(dc == NDC - 1),
                    )
                # V = Q + bias, in [V|1] (even h) / [1|V] (odd h) layout
                vv = v_sb[:, t].rearrange("p (pr a) f -> p pr (a f)", pr=HPC // 2)
                pv = ps[:, 0:OC].rearrange("p (pr c) -> p pr c", pr=HPC // 2)
                bv = bqk_rep[:, 0:OC].rearrange("p (pr c) -> p pr c", pr=HPC // 2)
                nc.vector.tensor_tensor(
                    vv[:, :, 0:DK], pv[:, :, 0:DK], bv[:, :, 0:DK], ALU.add
                )
                nc.vector.tensor_tensor(
                    vv[:, :, 3 * DK : 4 * DK],
                    pv[:, :, DK : 2 * DK],
                    bv[:, :, DK : 2 * DK],
                    ALU.add,
                )
                nc.vector.tensor_add(
                    k_sbs[tg][:, tl, :], ps[:, OC : 2 * OC], bqk_rep[:, OC : 2 * OC]
                )

            def rope(eng, src, dst, tg, pfx):
                """src/dst views [p, GT, n, DK]; cos/sin broadcast over dim 2."""
                n = src.shape[2]
                m = ropep.tile([128, GT, n, DK], BF16, tag=f"{pfx}m", name=f"{pfx}m{tg}")
                s = ropep.tile(
                    [128, GT, n, DK // 2], BF16, tag=f"{pfx}s", name=f"{pfx}s{tg}"
                )
                tsl = slice(tg * GT, (tg + 1) * GT)
                cos_bc = cos_sb[:, tsl].unsqueeze(2).to_broadcast((128, GT, n, DK))
                sin_bc = sin_sb[:, tsl].unsqueeze(2).to_broadcast((128, GT, n, DK // 2))
                x1 = src[:, :, :, 0 : DK // 2]
                x2 = src[:, :, :, DK // 2 : DK]
                eng.tensor_tensor(m[:], src, cos_bc, ALU.mult)
                eng.tensor_tensor(s[:], x2, sin_bc, ALU.mult)
                eng.tensor_tensor(
                    dst[:, :, :, 0 : DK // 2], m[:, :, :, 0 : DK // 2], s[:], ALU.subtract
                )
                eng.tensor_tensor(s[:], x1, sin_bc, ALU.mult)
                eng.tensor_tensor(
                    dst[:, :, :, DK // 2 : DK], m[:, :, :, DK // 2 : DK], s[:], ALU.add
                )

            def rope_q(tg):
                q_rope = ropep.tile(
                    [128, GT, HPC, DK], BF16, tag="q_rope", name=f"qr{tg}"
                )
                qv = q_rope[:].rearrange("p t (pr two) f -> p t pr (two f)", two=2)
                vv = v_sb[:, ts(tg, GT)].rearrange("p t (pr a) f -> p t pr (a f)", pr=2)
                rope(nc.vector, vv[:, :, :, 0:DK], qv[:, :, :, 0:DK], tg, "q")
                rope(nc.vector, vv[:, :, :, 3 * DK : 4 * DK], qv[:, :, :, DK : 2 * DK], tg, "q2")
                return q_rope

            def rope_k(tg, eng=None):
                k_rope = ropep.tile(
                    [128, GT, HPC, DK], BF16, tag="k_rope", name=f"kr{tg}"
                )
                k_view = k_sbs[tg][:].rearrange("p t (h f) -> p t h f", h=HPC)
                rope(eng or nc.vector, k_view, k_rope[:], tg, "k")
                return k_rope

            def tgroup(srcv, dst, tg, oc, cpeng):
                sv = srcv[:].rearrange("p t h f -> p t (h f)")
                tp = psA.tile([128, 512], BF16, tag="pp", name=f"tp{tg}{oc}")
                for tl in range(GT):
                    nc.tensor.transpose(
                        tp[:, ts(tl, 128)], sv[:, tl, ts(oc, 128)], ident[:]
                    )
                d_ = dst[oc * 2 + tg // 2][:, ts(tg % 2, 512)]
                if cpeng is nc.scalar:
                    cpeng.copy(d_, tp[:])
                else:
                    cpeng.tensor_copy(d_, tp[:])

            def proj_only(tg):
                k_sbs[tg] = ropep.tile([128, GT, OC], BF16, tag="k_sb", name=f"ks{tg}")
                for tl in range(GT):
                    proj_tile(tg, tl)


            # prep fillers (proj/rope/transpose of tg2,3) must ALL be emitted
            # during attention(0) — attention(1) reads their outputs from its
            # first matmul. oproj units are the only truly deferrable filler,
            # so they are reserved to plug attention(1)'s Act-bound gaps.
            prep_fillers = []
            oproj_fillers = []

            def drain_prep(n=1):
                for _ in range(min(n, len(prep_fillers))):
                    prep_fillers.pop(0)()

            def drain_oproj(n=1):
                for _ in range(min(n, len(oproj_fillers))):
                    oproj_fillers.pop(0)()

            def queue_tg(tg):
                # proj+rope first; the transposes are queued separately (by
                # queue_tg_transposes) so they drain several iterations after
                # the rope and never stall the in-order PE queue waiting on DVE
                holder = {}

                def mk_ksb():
                    k_sbs[tg] = ropep.tile(
                        [128, GT, OC], BF16, tag="k_sb", name=f"ks{tg}"
                    )

                prep_fillers.append(mk_ksb)
                for tl in range(GT):
                    prep_fillers.append(lambda tg=tg, tl=tl: proj_tile(tg, tl))

                def do_ropes(tg=tg):
                    holder["q"] = rope_q(tg)
                    holder["k"] = rope_k(tg)

                prep_fillers.append(do_ropes)
                return holder

            def queue_tg_transposes(tg, holder):
                for src, dst in (("q", qt_sb), ("k", kt_sb)):
                    for oc in range(2):
                        prep_fillers.append(
                            lambda src=src, dst=dst, tg=tg, oc=oc: tgroup(
                                holder[src], dst, tg, oc, nc.vector,
                            )
                        )

            def divide_chunk(h, c2, c, o2):
                # the replicated denominator sits on the 64 partitions opposite
                # head h's features: a cross-partition-base reciprocal moves it
                # onto the feature rows, then a lane-aligned elementwise multiply
                oc, ro = h // 2, 64 * (h % 2)
                rec = divp.tile([128, 512], F32, tag="rec", name=f"rc{c2}{h}{c}")
                nc.vector.reciprocal(rec[ds(ro, 64), :], o2[ds(64 - ro, 64), :])
                nc.vector.tensor_tensor(
                    heads_t[oc * 2 + c2][ds(ro, 64), ts(c - 2 * c2, 512)],
                    o2[ds(ro, 64), :],
                    rec[ds(ro, 64), :],
                    ALU.mult,
                )

            def oproj_chunk_units(c):
                # chunk 3 is the kernel tail: Act is idle there, so it takes
                # the PSUM->SBUF copies, and the out DMA is split per-half so
                # the last transfer is smaller
                c2 = c // 2
                units = []
                for tl in range(4):
                    t = 4 * c + tl
                    ot = outp.tile([128, D], F32, tag="ot", name=f"ot{t}")
                    for ic in range(2):
                        def unit(t=t, ic=ic, ot=ot, c2=c2, c=c):
                            po = psA.tile([128, 512], F32, tag="pp", name=f"po{t}{ic}")
                            for jc in range(2):
                                nc.tensor.matmul(
                                    po[:],
                                    heads_t[jc * 2 + c2][:, ds(128 * (t - 8 * c2), 128)],
                                    wo_sb[:, jc, ts(ic, 512)],
                                    start=(jc == 0),
                                    stop=(jc == 1),
                                )
                            if c == 3:
                                if (t + ic) % 2 == 0:
                                    nc.scalar.copy(ot[:, ts(ic, 512)], po[:])
                                else:
                                    nc.vector.tensor_copy(ot[:, ts(ic, 512)], po[:])
                                nc.sync.dma_start(
                                    out.ap()[ts(t, 128), ts(ic, 512)],
                                    ot[:, ts(ic, 512)],
                                )
                            else:
                                nc.vector.tensor_copy(ot[:, ts(ic, 512)], po[:])
                                if ic == 1:
                                    nc.sync.dma_start(out.ap()[ts(t, 128), :], ot[:])
                        units.append(unit)
                return units

            scale = float(1.0 / np.sqrt(DK))

            def attention(c2):
                # PE stream per step: sc(next step) is emitted BEFORE AV(this
                # step), so PE chews the next score matmul while Act runs exp.
                # Trailing narrow kt windows are GROUPED into one sc tile and
                # one exp: the sc-ring WAR loop (exp ack -> sc -> sem -> exp)
                # costs ~1us per traversal no matter how narrow the exp, so
                # merging two windows halves the latency-bound iterations.
                q0 = 1024 * c2
                if c2 == 0:
                    head_groups = [[0], [1], [2], [3], [4, 5], [6, 7]]
                else:
                    head_groups = [[kt] for kt in range(12)] + [[12, 13], [14, 15]]
                seq = [(h, tuple(g)) for h in range(HPC) for g in head_groups]
                scs = {}
                o2cs = {}

                def emit_sc(h, g):
                    oc, ro = h // 2, 64 * (h % 2)
                    qt_h = qt_sb[oc * 2 + c2][ds(ro, 64), :]
                    sc = psA.tile([128, 1024], F32, tag="sc", name=f"sc{c2}{h}{g[0]}")
                    offs = []
                    off = 0
                    for kt in g:
                        qs = max(q0, 128 * kt)
                        cw = q0 + 1024 - qs
                        for n5 in range((cw + 511) // 512):
                            ns = qs + 512 * n5
                            nw = min(512, q0 + 1024 - ns)
                            nc.tensor.matmul(
                                sc[:, ds(off + 512 * n5, nw)],
                                kt_sb[oc * 2 + kt // 8][ds(ro, 64), ts(kt % 8, 128)],
                                qt_h[:, ds(ns - q0, nw)],
                                start=True,
                                stop=True,
                            )
                        offs.append((kt, off, qs, cw))
                        off += cw
                    scs[(h, g)] = (sc, offs, off)

                emit_sc(*seq[0])
                for i, (h, g) in enumerate(seq):
                    if g[0] == 0:
                        o2cs[h] = {
                            c: psA.tile(
                                [128, 512], F32, tag="o2", name=f"o2_{c2}{h}{c}"
                            )
                            for c in (2 * c2, 2 * c2 + 1)
                        }
                    sc, offs, wtot = scs.pop((h, g))
                    et = expp.tile([128, 1024], BF16, tag="et", name=f"et{c2}{h}{g[0]}")
                    nc.scalar.activation(
                        et[:, ds(0, wtot)], sc[:, ds(0, wtot)], AF.Exp, scale=scale
                    )
                    if i + 1 < len(seq):
                        emit_sc(*seq[i + 1])
                    # fillers are placed BEFORE the exp-dependent AV: the AV
                    # waits on Act's exp anyway, so filler matmuls execute in
                    # that wait window instead of delaying the next score
                    # matmul (which would push the whole Act pipeline back)
                    drain_prep(1)
                    if c2 == 1:
                        if h == HPC - 1 and g[0] >= 12:
                            drain_oproj(3)
                        elif i % 3 == 0:
                            drain_oproj(1)
                    for kt, off, qs, cw in offs:
                        if qs == 128 * kt:  # zero the above-diagonal region
                            nc.gpsimd.tensor_tensor(
                                et[:, ds(off, 128)], et[:, ds(off, 128)],
                                maskt_sb[:], ALU.mult,
                            )
                    for kt, off, qs, cw in offs:
                        for c in (2 * c2, 2 * c2 + 1):
                            ce = 512 * (c + 1)
                            if ce <= qs:
                                continue
                            ns = max(qs, 512 * c)
                            nw = ce - ns
                            nc.tensor.matmul(
                                o2cs[h][c][:, ds(ns - 512 * c, nw)],
                                v_sb[:, kt, h, :],
                                et[:, ds(off + ns - qs, nw)],
                                start=(kt == 0),
                                stop=(kt == 4 * c + 3),
                            )
                        for c in (2 * c2, 2 * c2 + 1):
                            if kt == 4 * c + 3:
                                divide_chunk(h, c2, c, o2cs[h][c])
                                if h == HPC - 1:
                                    oproj_fillers.extend(oproj_chunk_units(c))

            # ---- emission ----
            # pre-attention software pipeline: PE runs proj(tg0), proj(tg1),
            # then the oc0 transposes, while DVE does bias+rope_q0+rope_q1 and
            # Pool ropes tg0's K in parallel. Only the oc0 (heads 0/1) halves
            # are transposed+copied up front — attention(0) starts with head 0;
            # the oc1 halves drain as the first prep fillers (needed by h2).
            proj_only(0)
            rq0 = rope_q(0)
            rk0 = rope_k(0, nc.gpsimd)
            proj_only(1)
            tgroup(rq0, qt_sb, 0, 0, nc.scalar)
            rq1 = rope_q(1)
            rk1 = rope_k(1, nc.vector)
            tgroup(rk0, kt_sb, 0, 0, nc.scalar)
            tgroup(rq1, qt_sb, 1, 0, nc.scalar)
            tgroup(rk1, kt_sb, 1, 0, nc.scalar)
            for srcv, dst, tg in ((rq0, qt_sb, 0), (rk0, kt_sb, 0),
                                  (rq1, qt_sb, 1), (rk1, kt_sb, 1)):
                prep_fillers.append(
                    lambda srcv=srcv, dst=dst, tg=tg: tgroup(
                        srcv, dst, tg, 1, nc.vector
                    )
                )
            h2 = queue_tg(2)
            h3 = queue_tg(3)
            queue_tg_transposes(2, h2)
            queue_tg_transposes(3, h3)
            attention(0)
            assert not prep_fillers, "prep fillers must drain inside attention(0)"
            attention(1)
            drain_oproj(len(oproj_fillers))

    nc.compile()
    return nc


_NC_CACHE = None


def _get_nc():
    global _NC_CACHE
    if _NC_CACHE is None:
        _NC_CACHE = build_kernel()
    return _NC_CACHE


_PERM = np.concatenate([np.arange(0, DK, 2), np.arange(1, DK, 2)])


def make_in_maps(in_features, token_positions, Wq, bq, Wk, bk, Wo):
    import ml_dtypes

    BF = ml_dtypes.bfloat16
    x = np.ascontiguousarray(np.asarray(in_features, dtype=np.float32))
    pos = np.asarray(token_positions, dtype=np.float32)
    Wq = np.asarray(Wq, dtype=np.float32)
    bq = np.asarray(bq, dtype=np.float32)
    Wk = np.asarray(Wk, dtype=np.float32)
    bk = np.asarray(bk, dtype=np.float32)
    Wo = np.asarray(Wo, dtype=np.float32)

    inv = (1.0 / THETA ** (np.arange(0, DK, 2, dtype=np.float32) / DK)).astype(
        np.float32
    )
    ang = pos[:, None] * inv[None, :]  # [T, 32]
    cos = np.cos(ang).astype(np.float32)
    sin = np.sin(ang).astype(np.float32)
    # table layout: [p, j, i] with token t = 128*j + p
    cosT = cos.reshape(NT, 128, DK // 2).transpose(1, 0, 2)
    cosT = np.ascontiguousarray(
        np.concatenate([cosT, cosT], axis=2).reshape(128, NT * DK)
    ).astype(BF)
    sinT = np.ascontiguousarray(
        sin.reshape(NT, 128, DK // 2).transpose(1, 0, 2).reshape(128, NT * (DK // 2))
    ).astype(BF)
    ii = np.arange(128)
    maskt = (ii[None, :] >= ii[:, None]).astype(BF)  # [k, q]: keep q >= k

    in_maps = []
    for c in range(NCORE):
        b, g = c // GPB, c % GPB
        cols = np.concatenate([DK * (HPC * g + hh) + _PERM for hh in range(HPC)])
        in_maps.append(
            {
                "xT": np.ascontiguousarray(x[b].T).astype(BF),
                "wqk": np.ascontiguousarray(
                    np.concatenate([Wq[cols].T, Wk[cols].T], axis=1)
                ).astype(BF),
                "wo": np.ascontiguousarray(Wo[:, cols].T).astype(BF),
                "bqk": np.ascontiguousarray(
                    np.concatenate([bq[cols], bk[cols]])[None, :]
                ),
                "cosT": cosT,
                "sinT": sinT,
                "maskt": maskt,
            }
        )
    return in_maps


def kernel(in_features, token_positions, Wq, bq, Wk, bk, Wv=None, bv=None, Wo=None, bo=None):
    from concourse import bass_utils

    nc = _get_nc()
    in_maps = make_in_maps(in_features, token_positions, Wq, bq, Wk, bk, Wo)
    res = bass_utils.run_bass_kernel_spmd(
        nc,
        in_maps,
        core_ids=list(range(NCORE)),
        trace=bool(int(os.environ.get("KERNEL_TRACE", "0"))),
    )
    outs = [res.results[c]["out"] for c in range(NCORE)]
    bo_f = np.asarray(bo, dtype=np.float32)
    full = np.stack(
        [np.sum(outs[b * GPB : (b + 1) * GPB], axis=0) + bo_f for b in range(B)]
    ).astype(np.float32)
    kernel.last_results = res
    return full


# revision 31
# speedup vs baseline: 1.0169x; 1.0169x over previous
"""Causal multi-head self-attention (RoPE, V-uses-Q-projection bug preserved)
as a Bass/Tile kernel for 8 Trainium2 NeuronCores — v4.

Sharding: core c -> batch b = c//4, head-group g = c%4 (4 heads of 16).
Each core computes its 4 heads' attention for its batch and a partial
output projection; partials are summed (and bo added) per batch on the host.

v4 changes over v3 (cost-model-guided; 190995ns -> 148197ns):
  - xt pool is 4-deep so no input DMA parks waiting on a ring buffer; late
    input DMAs ride the Pool queue so the Act SEQ stays free for the
    transpose copies that gate the first exp
  - PE p-state warmup: 64 dummy ident matmuls during the initial DMA wait
    ramp the tensor clock to 2.4GHz before the first projection matmul
  - pre-attention software pipeline: proj(tg0) / rope_q0+rope_k0 (DVE/Pool)
    / proj(tg1) / oc0-only transposes; the oc1 (heads 2/3) transposes defer
    into attention(0) as the first prep fillers
  - rope runs on DVE with bf16 cos/sin tables (2x_1p mode)
  - softmax divide: a cross-partition-base reciprocal moves the replicated
    denominator onto the feature rows in one DVE op (no partition-shift DMA)
  - attention emits sc(step i+1) BEFORE the exp-dependent AV(step i) so the
    in-order PE queue always has ready work while Act runs exp
  - two filler queues: prep (proj/rope/transpose of tg2,3 — prerequisites of
    attention(1), drained through attention(0) between sc and AV) and oproj
    (the only deferrable PE work, spread across attention(1)'s Act-bound
    stretches); transposes are queued well after their rope so they never
    head-of-line-block the PE queue
  - oproj chunk 3 (the kernel tail) copies PSUM->SBUF on the then-idle Act
    engine with per-half out DMAs; bo is added on the host
"""

import os
from contextlib import ExitStack

import numpy as np

import concourse.bass as bass
import concourse.mybir as mybir
import concourse.tile as tile
from concourse import bacc
from concourse.bass import ds, ts
from concourse.masks import make_identity

F32 = mybir.dt.float32
BF16 = mybir.dt.bfloat16
AF = mybir.ActivationFunctionType
ALU = mybir.AluOpType

B, T, D, H, DK = 2, 2048, 1024, 16, 64
THETA = 10000.0
NCORE, GPB = 8, 4          # cores; head-groups per batch
HPC = H // GPB             # heads per core = 4
OC = HPC * DK              # per-core projected features = 256
NT = T // 128              # 16 t-tiles
NDC = D // 128             # 8 contraction chunks
NG, GT = 4, 4              # t-groups; t-tiles per group


def build_kernel():
    nc = bacc.Bacc("TRN2", target_bir_lowering=False, debug=False)

    xT = nc.dram_tensor("xT", [D, T], BF16, kind="ExternalInput")
    wqk = nc.dram_tensor("wqk", [D, 2 * OC], BF16, kind="ExternalInput")
    wo = nc.dram_tensor("wo", [OC, D], BF16, kind="ExternalInput")
    bqk = nc.dram_tensor("bqk", [1, 2 * OC], F32, kind="ExternalInput")
    cosT = nc.dram_tensor("cosT", [128, NT * DK], BF16, kind="ExternalInput")
    sinT = nc.dram_tensor("sinT", [128, NT * (DK // 2)], BF16, kind="ExternalInput")
    maskt = nc.dram_tensor("maskt", [128, 128], BF16, kind="ExternalInput")
    out = nc.dram_tensor("out", [T, D], F32, kind="ExternalOutput")

    with tile.TileContext(nc) as tc, ExitStack() as top:
        consts = top.enter_context(tc.tile_pool(name="consts", bufs=1))
        wpool = top.enter_context(tc.tile_pool(name="weights", bufs=1))
        vk = top.enter_context(tc.tile_pool(name="vk", bufs=1))
        qtkt = top.enter_context(tc.tile_pool(name="qtkt", bufs=1))
        heads_pool = top.enter_context(tc.tile_pool(name="heads", bufs=1))

        # ---- input DMAs ----
        # coarse first transfers: proj tile 0 needs ALL dc chunks, and one
        # big DMA completes sooner than 8 serialized HWDGE generations
        wqk_sb = wpool.tile([128, NDC, 2 * OC], BF16)
        wqk_v = wqk.ap().rearrange("(dc p) c -> p dc c", p=128)
        nc.sync.dma_start(wqk_sb[:, 0:4, :], wqk_v[:, 0:4, :])
        nc.sync.dma_start(wqk_sb[:, 4:8, :], wqk_v[:, 4:8, :])

        xT_v = xT.ap().rearrange("(dc p) t -> p dc t", p=128)
        xtp = top.enter_context(tc.tile_pool(name="xt", bufs=4))
        xts = []

        def issue_xt(tg, eng, ndc_slice=1):
            t_ = xtp.tile([128, NDC, 512], BF16, tag="xt", name=f"xt{tg}")
            for s in range(ndc_slice):
                w = NDC // ndc_slice
                eng.dma_start(
                    t_[:, ds(s * w, w), :], xT_v[:, ds(s * w, w), ts(tg, 512)]
                )
            xts.append(t_)

        issue_xt(0, nc.scalar)
        bqk_rep = consts.tile([128, 2 * OC], F32)
        nc.scalar.dma_start(bqk_rep[:], bqk.ap().to_broadcast((128, 2 * OC)))
        issue_xt(1, nc.sync)

        # local compute while DMAs fly
        ident = consts.tile([128, 128], BF16)
        make_identity(nc, ident[:])
        v_sb = vk.tile([128, NT, HPC, 2 * DK], BF16)
        for h in range(HPC):  # ones on the side opposite the V features
            sl = slice(DK, 2 * DK) if h % 2 == 0 else slice(0, DK)
            nc.gpsimd.memset(v_sb[:, :, h, sl], 1.0)
        dume = consts.tile([1, 8], F32)
        nc.vector.memset(dume[:], 0.0)
        dume2 = consts.tile([1, 8], F32)
        nc.scalar.activation(dume2[:], dume[:], AF.Exp)  # preload Exp table

        # remaining inputs ride the Pool queue (dep-free, cheap to issue
        # there, and the Act SEQ stays free for the copies gating exp #0)
        cos_sb = consts.tile([128, NT, DK], BF16)
        nc.gpsimd.dma_start(cos_sb[:], cosT.ap().rearrange("p (j i) -> p j i", j=NT))
        sin_sb = consts.tile([128, NT, DK // 2], BF16)
        nc.gpsimd.dma_start(sin_sb[:], sinT.ap().rearrange("p (j i) -> p j i", j=NT))
        issue_xt(2, nc.gpsimd)
        maskt_sb = consts.tile([128, 128], BF16)
        nc.gpsimd.dma_start(maskt_sb[:], maskt.ap())
        issue_xt(3, nc.gpsimd)
        wo_sb = wpool.tile([128, 2, D], BF16)
        nc.gpsimd.dma_start(wo_sb[:], wo.ap().rearrange("(jc p) d -> p jc d", p=128))

        qt_sb = [
            qtkt.tile([128, T // 2], BF16, tag=f"qt{i}", name=f"qt_sb{i}")
            for i in range(4)
        ]
        kt_sb = [
            qtkt.tile([128, T // 2], BF16, tag=f"kt{i}", name=f"kt_sb{i}")
            for i in range(4)
        ]
        heads_t = [
            heads_pool.tile([128, T // 2], BF16, tag=f"ht{i}", name=f"heads_t{i}")
            for i in range(4)
        ]

        with tc.tile_pool(name="expp", bufs=int(os.environ.get("ET_BUFS", "10"))) as expp, \
             tc.tile_pool(name="divp", bufs=int(os.environ.get("DIV_BUFS", "4"))) as divp, \
             tc.tile_pool(name="outp", bufs=int(os.environ.get("OUTP_BUFS", "6"))) as outp, \
             tc.tile_pool(name="ropep", bufs=2) as ropep, \
             tc.tile_pool(name="psA", bufs=2, space="PSUM") as psA:

            k_sbs = {}

            # PE p-state warmup: the tensor engine only reaches 2.4GHz after
            # 3us of continuous execution (1.2GHz before, 0.65GHz from cold).
            # Dummy ident x ident matmuls during the initial DMA wait ramp the
            # clock so the DMA-paced projection matmuls run at full speed.
            # The warm tile rides the pp ring and is recycled by the first
            # projection tiles.
            warm = psA.tile([128, 512], F32, tag="pp", name="warm")
            for _ in range(64):
                nc.tensor.matmul(warm[:, 0:128], ident[:], ident[:],
                                 start=True, stop=True)

            def proj_tile(tg, tl):
                xt = xts[tg]
                t = GT * tg + tl
                ps = psA.tile([128, 2 * OC], F32, tag="pp", name=f"ps{t}")
                for dc in range(NDC):
                    nc.tensor.matmul(
                        ps[:],
                        xt[:, dc, ts(tl, 128)],
                        wqk_sb[:, dc, :],
                        start=(dc == 0),
                        stop=(dc == NDC - 1),
                    )
                # V = Q + bias, in [V|1] (even h) / [1|V] (odd h) layout
                vv = v_sb[:, t].rearrange("p (pr a) f -> p pr (a f)", pr=HPC // 2)
                pv = ps[:, 0:OC].rearrange("p (pr c) -> p pr c", pr=HPC // 2)
                bv = bqk_rep[:, 0:OC].rearrange("p (pr c) -> p pr c", pr=HPC // 2)
                nc.vector.tensor_tensor(
                    vv[:, :, 0:DK], pv[:, :, 0:DK], bv[:, :, 0:DK], ALU.add
                )
                nc.vector.tensor_tensor(
                    vv[:, :, 3 * DK : 4 * DK],
                    pv[:, :, DK : 2 * DK],
                    bv[:, :, DK : 2 * DK],
                    ALU.add,
                )
                nc.vector.tensor_add(
                    k_sbs[tg][:, tl, :], ps[:, OC : 2 * OC], bqk_rep[:, OC : 2 * OC]
                )

            def rope(eng, src, dst, tg, pfx):
                """src/dst views [p, GT, n, DK]; cos/sin broadcast over dim 2."""
                n = src.shape[2]
                m = ropep.tile([128, GT, n, DK], BF16, tag=f"{pfx}m", name=f"{pfx}m{tg}")
                s = ropep.tile(
                    [128, GT, n, DK // 2], BF16, tag=f"{pfx}s", name=f"{pfx}s{tg}"
                )
                tsl = slice(tg * GT, (tg + 1) * GT)
                cos_bc = cos_sb[:, tsl].unsqueeze(2).to_broadcast((128, GT, n, DK))
                sin_bc = sin_sb[:, tsl].unsqueeze(2).to_broadcast((128, GT, n, DK // 2))
                x1 = src[:, :, :, 0 : DK // 2]
                x2 = src[:, :, :, DK // 2 : DK]
                eng.tensor_tensor(m[:], src, cos_bc, ALU.mult)
                eng.tensor_tensor(s[:], x2, sin_bc, ALU.mult)
                eng.tensor_tensor(
                    dst[:, :, :, 0 : DK // 2], m[:, :, :, 0 : DK // 2], s[:], ALU.subtract
                )
                eng.tensor_tensor(s[:], x1, sin_bc, ALU.mult)
                eng.tensor_tensor(
                    dst[:, :, :, DK // 2 : DK], m[:, :, :, DK // 2 : DK], s[:], ALU.add
                )

            def rope_q(tg):
                q_rope = ropep.tile(
                    [128, GT, HPC, DK], BF16, tag="q_rope", name=f"qr{tg}"
                )
                qv = q_rope[:].rearrange("p t (pr two) f -> p t pr (two f)", two=2)
                vv = v_sb[:, ts(tg, GT)].rearrange("p t (pr a) f -> p t pr (a f)", pr=2)
                rope(nc.vector, vv[:, :, :, 0:DK], qv[:, :, :, 0:DK], tg, "q")
                rope(nc.vector, vv[:, :, :, 3 * DK : 4 * DK], qv[:, :, :, DK : 2 * DK], tg, "q2")
                return q_rope

            def rope_k(tg, eng=None):
                k_rope = ropep.tile(
                    [128, GT, HPC, DK], BF16, tag="k_rope", name=f"kr{tg}"
                )
                k_view = k_sbs[tg][:].rearrange("p t (h f) -> p t h f", h=HPC)
                rope(eng or nc.vector, k_view, k_rope[:], tg, "k")
                return k_rope

            def tgroup(srcv, dst, tg, oc, cpeng):
                sv = srcv[:].rearrange("p t h f -> p t (h f)")
                tp = psA.tile([128, 512], BF16, tag="pp", name=f"tp{tg}{oc}")
                for tl in range(GT):
                    nc.tensor.transpose(
                        tp[:, ts(tl, 128)], sv[:, tl, ts(oc, 128)], ident[:]
                    )
                d_ = dst[oc * 2 + tg // 2][:, ts(tg % 2, 512)]
                if cpeng is nc.scalar:
                    cpeng.copy(d_, tp[:])
                else:
                    cpeng.tensor_copy(d_, tp[:])

            def proj_only(tg):
                k_sbs[tg] = ropep.tile([128, GT, OC], BF16, tag="k_sb", name=f"ks{tg}")
                for tl in range(GT):
                    proj_tile(tg, tl)


            # prep fillers (proj/rope/transpose of tg2,3) must ALL be emitted
            # during attention(0) — attention(1) reads their outputs from its
            # first matmul. oproj units are the only truly deferrable filler,
            # so they are reserved to plug attention(1)'s Act-bound gaps.
            prep_fillers = []
            oproj_fillers = []

            def drain_prep(n=1):
                for _ in range(min(n, len(prep_fillers))):
                    prep_fillers.pop(0)()

            def drain_oproj(n=1):
                for _ in range(min(n, len(oproj_fillers))):
                    oproj_fillers.pop(0)()

            def queue_tg(tg):
                # proj+rope first; the transposes are queued separately (by
                # queue_tg_transposes) so they drain several iterations after
                # the rope and never stall the in-order PE queue waiting on DVE
                holder = {}

                def mk_ksb():
                    k_sbs[tg] = ropep.tile(
                        [128, GT, OC], BF16, tag="k_sb", name=f"ks{tg}"
                    )

                prep_fillers.append(mk_ksb)
                for tl in range(GT):
                    prep_fillers.append(lambda tg=tg, tl=tl: proj_tile(tg, tl))

                def do_ropes(tg=tg):
                    holder["q"] = rope_q(tg)
                    holder["k"] = rope_k(tg)

                prep_fillers.append(do_ropes)
                return holder

            def queue_tg_transposes(tg, holder):
                for src, dst in (("q", qt_sb), ("k", kt_sb)):
                    for oc in range(2):
                        prep_fillers.append(
                            lambda src=src, dst=dst, tg=tg, oc=oc: tgroup(
                                holder[src], dst, tg, oc, nc.vector,
                            )
                        )

            def divide_chunk(h, c2, c, o2):
                # the replicated denominator sits on the 64 partitions opposite
                # head h's features: a cross-partition-base reciprocal moves it
                # onto the feature rows, then a lane-aligned elementwise multiply
                oc, ro = h // 2, 64 * (h % 2)
                rec = divp.tile([128, 512], F32, tag="rec", name=f"rc{c2}{h}{c}")
                nc.vector.reciprocal(rec[ds(ro, 64), :], o2[ds(64 - ro, 64), :])
                nc.vector.tensor_tensor(
                    heads_t[oc * 2 + c2][ds(ro, 64), ts(c - 2 * c2, 512)],
                    o2[ds(ro, 64), :],
                    rec[ds(ro, 64), :],
                    ALU.mult,
                )

            def oproj_chunk_units(c):
                # chunk 3 is the kernel tail: Act is idle there, so it takes
                # the PSUM->SBUF copies, and the out DMA is split per-half so
                # the last transfer is smaller
                c2 = c // 2
                units = []
                for tl in range(4):
                    t = 4 * c + tl
                    ot = outp.tile([128, D], F32, tag="ot", name=f"ot{t}")
                    for ic in range(2):
                        def unit(t=t, ic=ic, ot=ot, c2=c2, c=c):
                            po = psA.tile([128, 512], F32, tag="pp", name=f"po{t}{ic}")
                            for jc in range(2):
                                nc.tensor.matmul(
                                    po[:],
                                    heads_t[jc * 2 + c2][:, ds(128 * (t - 8 * c2), 128)],
                                    wo_sb[:, jc, ts(ic, 512)],
                                    start=(jc == 0),
                                    stop=(jc == 1),
                                )
                            if c == 3:
                                if (t + ic) % 2 == 0:
                                    nc.scalar.copy(ot[:, ts(ic, 512)], po[:])
                                else:
                                    nc.vector.tensor_copy(ot[:, ts(ic, 512)], po[:])
                                nc.sync.dma_start(
                                    out.ap()[ts(t, 128), ts(ic, 512)],
                                    ot[:, ts(ic, 512)],
                                )
                            elif c == 2 and (t + ic) % 2 == 0:
                                nc.scalar.copy(ot[:, ts(ic, 512)], po[:])
                                if ic == 1:
                                    nc.sync.dma_start(out.ap()[ts(t, 128), :], ot[:])
                            else:
                                nc.vector.tensor_copy(ot[:, ts(ic, 512)], po[:])
                                if ic == 1:
                                    nc.sync.dma_start(out.ap()[ts(t, 128), :], ot[:])
                        units.append(unit)
                return units

            scale = float(1.0 / np.sqrt(DK))

            def attention(c2):
                # PE stream per step: sc(next step) is emitted BEFORE AV(this
                # step), so PE chews the next score matmul while Act runs exp —
                # the in-order PE queue never parks behind an exp-dependent AV.
                q0 = 1024 * c2
                nkt = 8 * (c2 + 1)
                seq = [(h, kt) for h in range(HPC) for kt in range(nkt)]
                scs = {}
                o2cs = {}

                def emit_sc(h, kt):
                    oc, ro = h // 2, 64 * (h % 2)
                    qt_h = qt_sb[oc * 2 + c2][ds(ro, 64), :]
                    qs = max(q0, 128 * kt)
                    cw = q0 + 1024 - qs
                    sc = psA.tile([128, 1024], F32, tag="sc", name=f"sc{c2}{h}{kt}")
                    for n5 in range((cw + 511) // 512):
                        ns = qs + 512 * n5
                        nw = min(512, q0 + 1024 - ns)
                        nc.tensor.matmul(
                            sc[:, ds(512 * n5, nw)],
                            kt_sb[oc * 2 + kt // 8][ds(ro, 64), ts(kt % 8, 128)],
                            qt_h[:, ds(ns - q0, nw)],
                            start=True,
                            stop=True,
                        )
                    scs[(h, kt)] = (sc, qs, cw)

                emit_sc(*seq[0])
                for i, (h, kt) in enumerate(seq):
                    if kt == 0:
                        o2cs[h] = {
                            c: psA.tile(
                                [128, 512], F32, tag="o2", name=f"o2_{c2}{h}{c}"
                            )
                            for c in (2 * c2, 2 * c2 + 1)
                        }
                    sc, qs, cw = scs.pop((h, kt))
                    et = expp.tile([128, 1024], BF16, tag="et", name=f"et{c2}{h}{kt}")
                    if i == 0 and c2 == 0 and cw > 512:
                        # split the very first exp: its low half only needs
                        # tg0's transposes, so the Act pipeline starts ~2us
                        # before tg1's q-copy lands
                        nc.scalar.activation(
                            et[:, ds(0, 512)], sc[:, ds(0, 512)], AF.Exp, scale=scale
                        )
                        nc.scalar.activation(
                            et[:, ds(512, cw - 512)], sc[:, ds(512, cw - 512)],
                            AF.Exp, scale=scale,
                        )
                    else:
                        nc.scalar.activation(
                            et[:, ds(0, cw)], sc[:, ds(0, cw)], AF.Exp, scale=scale
                        )
                    if i + 1 < len(seq):
                        emit_sc(*seq[i + 1])
                    # fillers are placed BEFORE the exp-dependent AV: the AV
                    # waits on Act's exp anyway, so filler matmuls execute in
                    # that wait window instead of delaying the next score
                    # matmul (which would push the whole Act pipeline back)
                    drain_prep(1)
                    if c2 == 1:
                        if h == HPC - 1 and kt >= 12:
                            drain_oproj(2)
                        elif i % 2 == 0:
                            drain_oproj(1)
                    if qs == 128 * kt:  # zero the above-diagonal region
                        nc.gpsimd.tensor_tensor(
                            et[:, 0:128], et[:, 0:128], maskt_sb[:], ALU.mult
                        )
                    for c in (2 * c2, 2 * c2 + 1):
                        ce = 512 * (c + 1)
                        if ce <= qs:
                            continue
                        ns = max(qs, 512 * c)
                        nw = ce - ns
                        nc.tensor.matmul(
                            o2cs[h][c][:, ds(ns - 512 * c, nw)],
                            v_sb[:, kt, h, :],
                            et[:, ds(ns - qs, nw)],
                            start=(kt == 0),
                            stop=(kt == 4 * c + 3),
                        )
                    for c in (2 * c2, 2 * c2 + 1):
                        if kt == 4 * c + 3:
                            divide_chunk(h, c2, c, o2cs[h][c])
                            if h == HPC - 1:
                                oproj_fillers.extend(oproj_chunk_units(c))

            # ---- emission ----
            # pre-attention software pipeline: PE runs proj(tg0), proj(tg1),
            # then the oc0 transposes, while DVE does bias+rope_q0+rope_q1 and
            # Pool ropes tg0's K in parallel. Only the oc0 (heads 0/1) halves
            # are transposed+copied up front — attention(0) starts with head 0;
            # the oc1 halves drain as the first prep fillers (needed by h2).
            proj_only(0)
            rq0 = rope_q(0)
            rk0 = rope_k(0, nc.gpsimd)
            proj_only(1)
            tgroup(rq0, qt_sb, 0, 0, nc.scalar)
            rq1 = rope_q(1)
            rk1 = rope_k(1, nc.vector)
            tgroup(rk0, kt_sb, 0, 0, nc.scalar)
            tgroup(rq1, qt_sb, 1, 0, nc.scalar)
            tgroup(rk1, kt_sb, 1, 0, nc.scalar)
            for srcv, dst, tg in ((rq0, qt_sb, 0), (rk0, kt_sb, 0),
                                  (rq1, qt_sb, 1), (rk1, kt_sb, 1)):
                prep_fillers.append(
                    lambda srcv=srcv, dst=dst, tg=tg: tgroup(
                        srcv, dst, tg, 1, nc.vector
                    )
                )
            h2 = queue_tg(2)
            h3 = queue_tg(3)
            queue_tg_transposes(2, h2)
            queue_tg_transposes(3, h3)
            attention(0)
            assert not prep_fillers, "prep fillers must drain inside attention(0)"
            attention(1)
            drain_oproj(len(oproj_fillers))

    nc.compile()
    return nc


_NC_CACHE = None


def _get_nc():
    global _NC_CACHE
    if _NC_CACHE is None:
        _NC_CACHE = build_kernel()
    return _NC_CACHE


_PERM = np.concatenate([np.arange(0, DK, 2), np.arange(1, DK, 2)])


def make_in_maps(in_features, token_positions, Wq, bq, Wk, bk, Wo):
    import ml_dtypes

    BF = ml_dtypes.bfloat16
    x = np.ascontiguousarray(np.asarray(in_features, dtype=np.float32))
    pos = np.asarray(token_positions, dtype=np.float32)
    Wq = np.asarray(Wq, dtype=np.float32)
    bq = np.asarray(bq, dtype=np.float32)
    Wk = np.asarray(Wk, dtype=np.float32)
    bk = np.asarray(bk, dtype=np.float32)
    Wo = np.asarray(Wo, dtype=np.float32)

    inv = (1.0 / THETA ** (np.arange(0, DK, 2, dtype=np.float32) / DK)).astype(
        np.float32
    )
    ang = pos[:, None] * inv[None, :]  # [T, 32]
    cos = np.cos(ang).astype(np.float32)
    sin = np.sin(ang).astype(np.float32)
    # table layout: [p, j, i] with token t = 128*j + p
    cosT = cos.reshape(NT, 128, DK // 2).transpose(1, 0, 2)
    cosT = np.ascontiguousarray(
        np.concatenate([cosT, cosT], axis=2).reshape(128, NT * DK)
    ).astype(BF)
    sinT = np.ascontiguousarray(
        sin.reshape(NT, 128, DK // 2).transpose(1, 0, 2).reshape(128, NT * (DK // 2))
    ).astype(BF)
    ii = np.arange(128)
    maskt = (ii[None, :] >= ii[:, None]).astype(BF)  # [k, q]: keep q >= k

    in_maps = []
    for c in range(NCORE):
        b, g = c // GPB, c % GPB
        cols = np.concatenate([DK * (HPC * g + hh) + _PERM for hh in range(HPC)])
        in_maps.append(
            {
                "xT": np.ascontiguousarray(x[b].T).astype(BF),
                "wqk": np.ascontiguousarray(
                    np.concatenate([Wq[cols].T, Wk[cols].T], axis=1)
                ).astype(BF),
                "wo": np.ascontiguousarray(Wo[:, cols].T).astype(BF),
                "bqk": np.ascontiguousarray(
                    np.concatenate([bq[cols], bk[cols]])[None, :]
                ),
                "cosT": cosT,
                "sinT": sinT,
                "maskt": maskt,
            }
        )
    return in_maps


def kernel(in_features, token_positions, Wq, bq, Wk, bk, Wv=None, bv=None, Wo=None, bo=None):
    from concourse import bass_utils

    nc = _get_nc()
    in_maps = make_in_maps(in_features, token_positions, Wq, bq, Wk, bk, Wo)
    res = bass_utils.run_bass_kernel_spmd(
        nc,
        in_maps,
        core_ids=list(range(NCORE)),
        trace=bool(int(os.environ.get("KERNEL_TRACE", "0"))),
    )
    outs = [res.results[c]["out"] for c in range(NCORE)]
    bo_f = np.asarray(bo, dtype=np.float32)
    full = np.stack(
        [np.sum(outs[b * GPB : (b + 1) * GPB], axis=0) + bo_f for b in range(B)]
    ).astype(np.float32)
    kernel.last_results = res
    return full
